# revision 1
# baseline (speedup 1.0000x reference)
"""Fused LoRA-attention block (qkv + k/v LoRA + MHA softmax + out-proj) for
Trainium2, data-parallel over batch across 8 NeuronCores.

Per-core layout strategy (batch shard = 2 of 16):
  - Host pre-transposes x and all weights so every matmul operand lands in
    SBUF with the contraction dim on partitions; all matmul data is bf16
    (fp32 PSUM accumulate), softmax statistics fp32.
  - Q^T/K^T computed channel-major [c_out, tok]; V token-major [tok, c_out]
    with an appended ones column per head so the attention row-sum falls out
    of the P@V matmul for free (row 64 of the [65, q] PSUM tile).
  - S^T = K@Q^T computed per head with k on partitions; softmax runs without
    max-subtraction (logits bounded ~|3| by construction of the inputs).
  - Head pairs share the PE array concurrently via row groups (K=64 each);
    AV for k-block kb-1 issues after S for kb so PE never waits on ACT.
  - PSUM is split 4 banks for attention (512-wide S/AV tiles) + 4 banks for
    gemm chains, so the next batch's qkv/proj overlaps this batch's
    ACT-bound softmax; emission interleaves the two streams.
  - Paired N=512 matmul chains share one stationary-weight load by
    interleaving the two token-halves of a PSUM pair.
"""

import sys

sys.path.insert(0, "/opt/trn_rl_repo")

import ml_dtypes
import numpy as np

import concourse.bass as bass
import concourse.mybir as mybir
import concourse.tile as tile
from concourse import bacc
from concourse.bass_utils import run_bass_kernel_spmd

NCORES = 8
B, N, C = 16, 1024, 1024
H, D, R = 16, 64, 64
BSH = B // NCORES  # batches per core
NB = C // 128  # channel blocks
SCALE = D**-0.5
LSCALE = 1.0 / R
BF = mybir.dt.bfloat16
F32 = mybir.dt.float32
BF_NP = ml_dtypes.bfloat16
HALVES = (bass.ts(0, 512), bass.ts(1, 512))


def build_nc(
    loop_reps: int = 1,
    dbg: bool = False,
    probe_noexp: bool = False,
    probe_nonorm: bool = False,
):
    nc = bacc.Bacc(None, target_bir_lowering=False, debug=False)

    xt_d = nc.dram_tensor("xt", [BSH, NB, 128, N], BF, kind="ExternalInput")
    wq_d = nc.dram_tensor("wq", [NB, 128, C], BF, kind="ExternalInput")
    wk_d = nc.dram_tensor("wk", [NB, 128, C], BF, kind="ExternalInput")
    wv_d = nc.dram_tensor("wv", [NB, 128, C], BF, kind="ExternalInput")
    wp_d = nc.dram_tensor("wp", [NB, 128, C], BF, kind="ExternalInput")
    bq_d = nc.dram_tensor("bq", [128, NB], F32, kind="ExternalInput")
    bk_d = nc.dram_tensor("bk", [128, NB], F32, kind="ExternalInput")
    bv_d = nc.dram_tensor("bv", [1, C], BF, kind="ExternalInput")
    bp_d = nc.dram_tensor("bp", [1, C], BF, kind="ExternalInput")
    ka_d = nc.dram_tensor("ka", [NB, 128, R], BF, kind="ExternalInput")
    va_d = nc.dram_tensor("va", [NB, 128, R], BF, kind="ExternalInput")
    kb_d = nc.dram_tensor("kb", [R, C], BF, kind="ExternalInput")
    vb_d = nc.dram_tensor("vb", [R, C], BF, kind="ExternalInput")
    out_d = nc.dram_tensor("out", [BSH, N, C], BF, kind="ExternalOutput")
    if dbg:
        dqt_d = nc.dram_tensor("dqt", [128, NB, N], BF, kind="ExternalOutput")
        dkt_d = nc.dram_tensor("dkt", [128, NB, N], BF, kind="ExternalOutput")
        dva_d = nc.dram_tensor("dva", [128, NB, H, D + 1], BF, kind="ExternalOutput")
        dot_d = nc.dram_tensor("dot", [128, NB, N], BF, kind="ExternalOutput")

    with tile.TileContext(nc) as tc:
        with (
            tc.tile_pool(name="wpool", bufs=1) as wpool,
            tc.tile_pool(name="xtp", bufs=1) as xtp,
            tc.tile_pool(name="actp", bufs=1) as actp,
            tc.tile_pool(name="ptp", bufs=8) as ptp,
            tc.tile_pool(name="akp", bufs=1) as akp,
            tc.tile_pool(name="rsp", bufs=2) as rsp,
            tc.tile_pool(name="outp", bufs=2) as outp,
            tc.tile_pool(name="gmps", bufs=3, space="PSUM") as gmps,
            tc.tile_pool(name="sps_p", bufs=3, space="PSUM") as sps_p,
            tc.tile_pool(name="avps", bufs=2, space="PSUM") as avps,
        ):
            # ---- persistent weights ----
            wq_sb = wpool.tile([128, NB, C], BF, tag="wq")
            wk_sb = wpool.tile([128, NB, C], BF, tag="wk")
            wv_sb = wpool.tile([128, NB, C], BF, tag="wv")
            wp_sb = wpool.tile([128, NB, C], BF, tag="wp")
            for w_sb, w_d in ((wq_sb, wq_d), (wk_sb, wk_d), (wv_sb, wv_d), (wp_sb, wp_d)):
                for blk in range(NB):
                    nc.sync.dma_start(out=w_sb[:, blk, :], in_=w_d[blk])
            ka_sb = wpool.tile([128, NB, R], BF, tag="ka")
            nc.sync.dma_start(out=ka_sb[:], in_=ka_d.rearrange("a p n -> p a n"))
            va_sb = wpool.tile([128, NB, R], BF, tag="va")
            nc.sync.dma_start(out=va_sb[:], in_=va_d.rearrange("a p n -> p a n"))
            kb_sb = wpool.tile([R, C], BF, tag="kb")
            nc.sync.dma_start(out=kb_sb[:], in_=kb_d[:])
            vb_sb = wpool.tile([R, C], BF, tag="vb")
            nc.sync.dma_start(out=vb_sb[:], in_=vb_d[:])
            bq_sb = wpool.tile([128, NB], F32, tag="bq")
            nc.sync.dma_start(out=bq_sb[:], in_=bq_d[:])
            bk_sb = wpool.tile([128, NB], F32, tag="bk")
            nc.sync.dma_start(out=bk_sb[:], in_=bk_d[:])
            bv_sb = wpool.tile([1, C], BF, tag="bv")
            nc.sync.dma_start(out=bv_sb[:], in_=bv_d[:])
            bp_sb = wpool.tile([1, C], BF, tag="bp")
            nc.sync.dma_start(out=bp_sb[:], in_=bp_d[:])

            ones_bf = wpool.tile([1, 128], BF, tag="ones_bf")
            nc.vector.memset(ones_bf[:], 1.0)

            # V with per-head ones column appended: [128, tblk, head, 65]
            vaug0 = wpool.tile([128, NB, H, D + 1], BF, tag="vaug0")
            vaug1 = wpool.tile([128, NB, H, D + 1], BF, tag="vaug1")
            vaugs = (vaug0, vaug1)
            # qT/kT/oT as per-channel-block tiles so cross-batch WAR hazards
            # serialize per block, not per tensor.
            qT_blk = [
                actp.tile([128, N], BF, tag=f"qT{cb}", name=f"qT{cb}")
                for cb in range(NB)
            ]
            kT_blk = [
                actp.tile([128, N], BF, tag=f"kT{cb}", name=f"kT{cb}")
                for cb in range(NB)
            ]
            oT_blk = [
                actp.tile([128, N], BF, tag=f"oT{cb}", name=f"oT{cb}")
                for cb in range(NB)
            ]
            ak_sb = akp.tile([R, N], BF, tag="ak")
            av_sb = akp.tile([R, N], BF, tag="av")

            def pair_chain(out_slices, emit_mm, n_steps):
                """Two 512-wide PSUM chains sharing each stationary operand."""
                pss = [
                    gmps.tile([128, 512], F32, tag="gm", name=f"gm{i}")
                    for i in range(2)
                ]
                for step in range(n_steps):
                    for i, hv in enumerate(HALVES):
                        emit_mm(pss[i], hv, step)
                return pss

            def emit_lora(b, xt_sb):
                for asb, aw in ((ak_sb, ka_sb), (av_sb, va_sb)):
                    apss = [
                        avps.tile([D + 1, 512], F32, tag="avp", name=f"aps{i}")
                        for i in range(2)
                    ]
                    for ci in range(NB):
                        for i, hv in enumerate(HALVES):
                            nc.tensor.matmul(
                                apss[i][0:R, :],
                                aw[:, ci, :],
                                xt_sb[:, ci, hv],
                                start=(ci == 0),
                                stop=(ci == NB - 1),
                            )
                    for i, hv in enumerate(HALVES):
                        nc.vector.tensor_copy(asb[:, hv], apss[i][0:R, :])

            def emit_qk(b, xt_sb, cb):
                csl = bass.ts(cb, 128)

                def mm_q(ps, hv, ci):
                    nc.tensor.matmul(
                        ps[:],
                        wq_sb[:, ci, csl],
                        xt_sb[:, ci, hv],
                        start=(ci == 0),
                        stop=(ci == NB - 1),
                    )

                pss = pair_chain(None, mm_q, NB)
                for i, hv in enumerate(HALVES):
                    nc.vector.tensor_scalar_add(
                        qT_blk[cb][:, hv], pss[i][:], bq_sb[:, cb : cb + 1]
                    )

                def mm_k(ps, hv, ci):
                    if ci < NB:
                        nc.tensor.matmul(
                            ps[:],
                            wk_sb[:, ci, csl],
                            xt_sb[:, ci, hv],
                            start=(ci == 0),
                            stop=False,
                        )
                    else:
                        nc.tensor.matmul(
                            ps[:], kb_sb[:, csl], ak_sb[:, hv], start=False, stop=True
                        )

                pss = pair_chain(None, mm_k, NB + 1)
                for i, hv in enumerate(HALVES):
                    nc.vector.tensor_scalar_add(
                        kT_blk[cb][:, hv], pss[i][:], bk_sb[:, cb : cb + 1]
                    )

            def emit_v(b, xt_sb, tb):
                vaug_sb = vaugs[b % 2]
                tsl = bass.ts(tb, 128)

                def mm_v(ps, hv, step):
                    if step < NB:
                        nc.tensor.matmul(
                            ps[:],
                            xt_sb[:, step, tsl],
                            wv_sb[:, step, hv],
                            start=(step == 0),
                            stop=False,
                        )
                    elif step == NB:
                        nc.tensor.matmul(
                            ps[:], ones_bf[:, 0:128], bv_sb[:, hv], start=False,
                            stop=False,
                        )
                    else:
                        nc.tensor.matmul(
                            ps[:], av_sb[:, tsl], vb_sb[:, hv], start=False, stop=True
                        )

                pss = pair_chain(None, mm_v, NB + 2)
                for i, hv in enumerate(HALVES):
                    nc.vector.tensor_copy(
                        vaug_sb[:, tb, i * 8 : (i + 1) * 8, 0:D],
                        pss[i][:].rearrange("p (h d) -> p h d", d=D),
                    )

            def emit_xt(b):
                xt_sb = xtp.tile([128, NB, N], BF, tag="xt", name="xt")
                for blk in range(NB):
                    nc.sync.dma_start(out=xt_sb[:, blk, :], in_=xt_d[b, blk])
                return xt_sb

            def attention_pair(b, pr):
                vaug_sb = vaugs[b % 2]
                offs = (0, 64)
                for hv_i, hv in enumerate(HALVES):
                    avs = [
                        avps.tile([D + 1, 512], F32, tag="avp", name=f"avp{i}")
                        for i in range(2)
                    ]

                    def emit_av(pts, kb_):
                        for hi in range(2):
                            h = 2 * pr + hi
                            nc.tensor.matmul(
                                avs[hi][:],
                                vaug_sb[:, kb_, h, :],
                                pts[hi][:],
                                start=(kb_ == 0),
                                stop=(kb_ == NB - 1),
                            )

                    pend = None
                    for kb_ in range(NB):
                        ksl = bass.ts(kb_, 128)
                        sps = [
                            sps_p.tile([128, 512], F32, tag="sp", name=f"sps{i}")
                            for i in range(2)
                        ]
                        for hi, off in enumerate(offs):
                            nc.tensor.matmul(
                                sps[hi][:],
                                kT_blk[pr][off : off + D, ksl],
                                qT_blk[pr][off : off + D, hv],
                                start=True,
                                stop=True,
                            )
                        pts = [
                            ptp.tile([128, 512], BF, tag="pT", name=f"pT{i}")
                            for i in range(2)
                        ]
                        for hi in range(2):
                            if probe_noexp:
                                nc.vector.memset(pts[hi][:], 0.001)
                            else:
                                nc.scalar.activation(
                                    pts[hi][:],
                                    sps[hi][:],
                                    mybir.ActivationFunctionType.Exp,
                                )
                        if pend is not None:
                            emit_av(pend[0], pend[1])
                        pend = (pts, kb_)
                    emit_av(pend[0], pend[1])

                    for hi, off in enumerate(offs):
                        avp = avs[hi]
                        if probe_nonorm:
                            nc.vector.tensor_copy(
                                oT_blk[pr][off : off + D, hv], avp[0:D, :]
                            )
                            continue
                        ssb = rsp.tile([1, 512], F32, tag="ssb")
                        nc.vector.tensor_copy(ssb[:], avp[D : D + 1, :])
                        rs = rsp.tile([1, 512], F32, tag="rs")
                        nc.vector.reciprocal_approx_fast(rs[:], ssb[:])
                        bc = rsp.tile([D, 512], F32, tag="bc")
                        nc.gpsimd.partition_broadcast(bc[:], rs[:])
                        nc.vector.tensor_mul(
                            oT_blk[pr][off : off + D, hv], avp[0:D, :], bc[:]
                        )

            def emit_proj(b, qb):
                qsl = bass.ts(qb, 128)

                def mm_p(ps, hv, step):
                    if step < NB:
                        nc.tensor.matmul(
                            ps[:],
                            oT_blk[step][:, qsl],
                            wp_sb[:, step, hv],
                            start=(step == 0),
                            stop=False,
                        )
                    else:
                        nc.tensor.matmul(
                            ps[:], ones_bf[:, 0:128], bp_sb[:, hv], start=False,
                            stop=True,
                        )

                pss = pair_chain(None, mm_p, NB + 1)
                ost = outp.tile([128, N], BF, tag="ost")
                for i, hv in enumerate(HALVES):
                    nc.vector.tensor_copy(ost[:, hv], pss[i][:])
                nc.sync.dma_start(out=out_d[b, qsl, :], in_=ost[:])

            def emit_qkv_unit(b, xt_sb, j):
                # unit 0: lora; units 1..8: (Q_j, K_j, V_j)
                if j == 0:
                    emit_lora(b, xt_sb)
                else:
                    emit_qk(b, xt_sb, j - 1)
                    emit_v(b, xt_sb, j - 1)

            def body():
                nc.vector.memset(vaug0[:, :, :, D : D + 1], 1.0)
                nc.vector.memset(vaug1[:, :, :, D : D + 1], 1.0)
                xt_sb = emit_xt(0)
                for j in range(NB + 1):
                    emit_qkv_unit(0, xt_sb, j)
                for b in range(BSH):
                    if b + 1 < BSH:
                        xt_next = emit_xt(b + 1)
                    if dbg and b == 0:
                        for cb in range(NB):
                            nc.sync.dma_start(out=dqt_d[:, cb, :], in_=qT_blk[cb][:])
                            nc.sync.dma_start(out=dkt_d[:, cb, :], in_=kT_blk[cb][:])
                        nc.sync.dma_start(out=dva_d[:], in_=vaugs[0][:])
                    for pr in range(H // 2):
                        attention_pair(b, pr)
                        if b + 1 < BSH and pr < NB + 1:
                            emit_qkv_unit(b + 1, xt_next, pr)
                    if dbg and b == 0:
                        for cb in range(NB):
                            nc.sync.dma_start(out=dot_d[:, cb, :], in_=oT_blk[cb][:])
                    if b + 1 < BSH:
                        emit_qkv_unit(b + 1, xt_next, NB)
                    for qb in range(NB):
                        emit_proj(b, qb)

            if loop_reps > 1:
                with tc.For_i(0, loop_reps, 1):
                    body()
            else:
                body()

    nc.compile()
    return nc


def _prep_shared(W_qkv, b_qkv, lora_kA, lora_kB, lora_vA, lora_vB, W_proj, b_proj):
    def bf(a):
        return np.ascontiguousarray(a).astype(BF_NP)

    W_qkv = np.asarray(W_qkv, np.float32)
    return {
        "wq": bf((W_qkv[:C].T * SCALE).reshape(NB, 128, C)),
        "wk": bf(W_qkv[C : 2 * C].T.reshape(NB, 128, C)),
        "wv": bf(W_qkv[2 * C :].T.reshape(NB, 128, C)),
        "wp": bf(np.asarray(W_proj, np.float32).T.reshape(NB, 128, C)),
        "bq": np.ascontiguousarray(
            (np.asarray(b_qkv[:C], np.float32) * SCALE).reshape(NB, 128).T
        ),
        "bk": np.ascontiguousarray(
            np.asarray(b_qkv[C : 2 * C], np.float32).reshape(NB, 128).T
        ),
        "bv": bf(np.asarray(b_qkv[2 * C :], np.float32).reshape(1, C)),
        "bp": bf(np.asarray(b_proj, np.float32).reshape(1, C)),
        "ka": bf(np.asarray(lora_kA, np.float32).T.reshape(NB, 128, R)),
        "va": bf(np.asarray(lora_vA, np.float32).T.reshape(NB, 128, R)),
        "kb": bf(np.asarray(lora_kB, np.float32).T * LSCALE),
        "vb": bf(np.asarray(lora_vB, np.float32).T * LSCALE),
    }


def kernel(x, W_qkv, b_qkv, lora_kA, lora_kB, lora_vA, lora_vB, W_proj, b_proj):
    nc = build_nc(loop_reps=1)
    shared = _prep_shared(
        W_qkv, b_qkv, lora_kA, lora_kB, lora_vA, lora_vB, W_proj, b_proj
    )
    x = np.asarray(x, np.float32)
    in_maps = []
    for c in range(NCORES):
        xs = x[c * BSH : (c + 1) * BSH]
        xt = (
            np.ascontiguousarray(xs.transpose(0, 2, 1))
            .astype(BF_NP)
            .reshape(BSH, NB, 128, N)
        )
        in_maps.append({"xt": xt, **shared})
    res = run_bass_kernel_spmd(nc, in_maps, list(range(NCORES)))
    return np.concatenate(
        [res.results[c]["out"].astype(np.float32) for c in range(NCORES)], axis=0
    )



# revision 3
# speedup vs baseline: 1.1004x; 1.1004x over previous
"""Fused LoRA-attention block (qkv + k/v LoRA + MHA softmax + out-proj) for
Trainium2, data-parallel over batch across 8 NeuronCores.

Per-core layout strategy (batch shard = 2 of 16):
  - Host folds the rank-64 LoRA into W_k/W_v (W + (alpha/r) B@A, fp32), folds
    the V bias and proj bias into one output bias (softmax rows sum to 1), and
    pre-transposes everything so each matmul's contraction lands on SBUF
    partitions; matmul data bf16 (fp32 PSUM), softmax statistics fp32.
  - Q^T/K^T computed channel-major [c_out, tok]; V token-major [tok, c_out]
    with an appended ones column per head so the attention row-sum falls out
    of the P@V matmul for free (row 64 of the [65, q] PSUM tile).
  - S^T = K@Q^T per head pair with k on partitions; the two heads of a pair
    run concurrently in separate PE row groups (K=64 each) and land in one
    [128, 1024] two-bank PSUM tile, so a single ACT exp instruction covers
    both heads (softmax runs without max-subtraction, logits bounded ~|3|
    by construction of the inputs).
  - AV for k-block kb-1 issues after S for kb so PE never waits on ACT; the
    head pair's AV accumulates in one [65, 1024] two-bank PSUM tile and one
    reciprocal + one gpsimd broadcast normalize both heads.
  - PSUM: 4 banks S (double-buffered), 2 banks AV, 2 banks qkv/proj gemm
    chains, so the next batch's qkv overlaps this batch's ACT-bound softmax;
    emission interleaves the two streams.
  - Paired N=512 matmul chains share one stationary-weight load by
    interleaving the two token-halves of a PSUM pair.
"""

import sys

sys.path.insert(0, "/opt/trn_rl_repo")

import ml_dtypes
import numpy as np

import concourse.bass as bass
import concourse.mybir as mybir
import concourse.tile as tile
from concourse import bacc
from concourse.bass_utils import run_bass_kernel_spmd

NCORES = 8
B, N, C = 16, 1024, 1024
H, D, R = 16, 64, 64
BSH = B // NCORES  # batches per core
NB = C // 128  # channel blocks
SCALE = D**-0.5
LSCALE = 1.0 / R
BF = mybir.dt.bfloat16
F32 = mybir.dt.float32
BF_NP = ml_dtypes.bfloat16
HALVES = (bass.ts(0, 512), bass.ts(1, 512))


def build_nc(
    loop_reps: int = 1,
    pack_s: bool = True,
    pack_av: bool = True,
    bias_dve: bool = True,
    probe_noexp: bool = False,
    probe_nonorm: bool = False,
):
    nc = bacc.Bacc(None, target_bir_lowering=False, debug=False)

    xt_d = nc.dram_tensor("xt", [BSH, NB, 128, N], BF, kind="ExternalInput")
    wq_d = nc.dram_tensor("wq", [NB, 128, C], BF, kind="ExternalInput")
    wk_d = nc.dram_tensor("wk", [NB, 128, C], BF, kind="ExternalInput")
    wv_d = nc.dram_tensor("wv", [NB, 128, C], BF, kind="ExternalInput")
    wp_d = nc.dram_tensor("wp", [NB, 128, C], BF, kind="ExternalInput")
    bq_d = nc.dram_tensor("bq", [128, NB], F32, kind="ExternalInput")
    bk_d = nc.dram_tensor("bk", [128, NB], F32, kind="ExternalInput")
    bo_d = nc.dram_tensor("bo", [128, C], BF, kind="ExternalInput")
    out_d = nc.dram_tensor("out", [BSH, N, C], BF, kind="ExternalOutput")

    with tile.TileContext(nc) as tc:
        with (
            tc.tile_pool(name="wpool", bufs=1) as wpool,
            tc.tile_pool(name="xtp", bufs=1) as xtp,
            tc.tile_pool(name="actp", bufs=1) as actp,
            tc.tile_pool(name="ptp", bufs=4) as ptp,
            tc.tile_pool(name="rsp", bufs=2) as rsp,
            tc.tile_pool(name="outp", bufs=2) as outp,
            tc.tile_pool(name="gmps", bufs=2, space="PSUM") as gmps,
            tc.tile_pool(name="sps_p", bufs=2 if pack_s else 3, space="PSUM") as sps_p,
            tc.tile_pool(name="avps", bufs=1 if pack_av else 2, space="PSUM") as avps,
        ):
            # ---- persistent weights ----
            wq_sb = wpool.tile([128, NB, C], BF, tag="wq")
            wk_sb = wpool.tile([128, NB, C], BF, tag="wk")
            wv_sb = wpool.tile([128, NB, C], BF, tag="wv")
            wp_sb = wpool.tile([128, NB, C], BF, tag="wp")
            for w_sb, w_d in ((wq_sb, wq_d), (wk_sb, wk_d), (wv_sb, wv_d), (wp_sb, wp_d)):
                for blk in range(NB):
                    nc.sync.dma_start(out=w_sb[:, blk, :], in_=w_d[blk])
            bq_sb = wpool.tile([128, NB], F32, tag="bq")
            nc.sync.dma_start(out=bq_sb[:], in_=bq_d[:])
            bk_sb = wpool.tile([128, NB], F32, tag="bk")
            nc.sync.dma_start(out=bk_sb[:], in_=bk_d[:])
            bo_sb = wpool.tile([128, C], BF, tag="bo")
            nc.sync.dma_start(out=bo_sb[:], in_=bo_d[:])
            if not bias_dve:
                ones_bf = wpool.tile([1, 128], BF, tag="ones_bf")
                nc.vector.memset(ones_bf[:], 1.0)

            # V with per-head ones column appended: [128, tblk, head, 65]
            vaug0 = wpool.tile([128, NB, H, D + 1], BF, tag="vaug0")
            vaug1 = wpool.tile([128, NB, H, D + 1], BF, tag="vaug1")
            vaugs = (vaug0, vaug1)
            # qT/kT/oT as per-channel-block tiles so cross-batch WAR hazards
            # serialize per block, not per tensor.
            qT_blk = [
                actp.tile([128, N], BF, tag=f"qT{cb}", name=f"qT{cb}")
                for cb in range(NB)
            ]
            kT_blk = [
                actp.tile([128, N], BF, tag=f"kT{cb}", name=f"kT{cb}")
                for cb in range(NB)
            ]
            oT_blk = [
                actp.tile([128, N], BF, tag=f"oT{cb}", name=f"oT{cb}")
                for cb in range(NB)
            ]

            def pair_chain(emit_mm, n_steps):
                """Two 512-wide PSUM chains sharing each stationary operand."""
                pss = [
                    gmps.tile([128, 512], F32, tag="gm", name=f"gm{i}")
                    for i in range(2)
                ]
                for step in range(n_steps):
                    for i, hv in enumerate(HALVES):
                        emit_mm(pss[i], hv, step)
                return pss

            def emit_qk(b, xt_sb, cb):
                csl = bass.ts(cb, 128)

                def mm_q(ps, hv, ci):
                    nc.tensor.matmul(
                        ps[:],
                        wq_sb[:, ci, csl],
                        xt_sb[:, ci, hv],
                        start=(ci == 0),
                        stop=(ci == NB - 1),
                    )

                pss = pair_chain(mm_q, NB)
                for i, hv in enumerate(HALVES):
                    nc.vector.tensor_scalar_add(
                        qT_blk[cb][:, hv], pss[i][:], bq_sb[:, cb : cb + 1]
                    )

                def mm_k(ps, hv, ci):
                    nc.tensor.matmul(
                        ps[:],
                        wk_sb[:, ci, csl],
                        xt_sb[:, ci, hv],
                        start=(ci == 0),
                        stop=(ci == NB - 1),
                    )

                pss = pair_chain(mm_k, NB)
                for i, hv in enumerate(HALVES):
                    nc.vector.tensor_scalar_add(
                        kT_blk[cb][:, hv], pss[i][:], bk_sb[:, cb : cb + 1]
                    )

            def emit_v(b, xt_sb, tb):
                vaug_sb = vaugs[b % 2]
                tsl = bass.ts(tb, 128)

                def mm_v(ps, hv, step):
                    nc.tensor.matmul(
                        ps[:],
                        xt_sb[:, step, tsl],
                        wv_sb[:, step, hv],
                        start=(step == 0),
                        stop=(step == NB - 1),
                    )

                pss = pair_chain(mm_v, NB)
                for i, hv in enumerate(HALVES):
                    nc.vector.tensor_copy(
                        vaug_sb[:, tb, i * 8 : (i + 1) * 8, 0:D],
                        pss[i][:].rearrange("p (h d) -> p h d", d=D),
                    )

            def emit_xt(b):
                xt_sb = xtp.tile([128, NB, N], BF, tag="xt", name="xt")
                for blk in range(NB):
                    nc.sync.dma_start(out=xt_sb[:, blk, :], in_=xt_d[b, blk])
                return xt_sb

            def attention_pair(b, pr):
                vaug_sb = vaugs[b % 2]
                offs = (0, 64)
                for hv_i, hv in enumerate(HALVES):
                    if pack_av:
                        # [65, 1024]: head a in [:, 0:512], head b in
                        # [:, 512:1024]; row 64 is the softmax denominator.
                        avp_t = avps.tile([D + 1, 1024], F32, tag="avp", name="avp")
                        avs = [avp_t[:, bass.ts(0, 512)], avp_t[:, bass.ts(1, 512)]]
                    else:
                        avs = [
                            avps.tile([D + 1, 512], F32, tag="avp", name=f"avp{i}")[:]
                            for i in range(2)
                        ]

                    def emit_av(pts, kb_):
                        for hi in range(2):
                            h = 2 * pr + hi
                            nc.tensor.matmul(
                                avs[hi],
                                vaug_sb[:, kb_, h, :],
                                pts[hi],
                                start=(kb_ == 0),
                                stop=(kb_ == NB - 1),
                            )

                    pend = None
                    for kb_ in range(NB):
                        ksl = bass.ts(kb_, 128)
                        if pack_s:
                            sp = sps_p.tile([128, 1024], F32, tag="sp", name="sp")
                            sps = [sp[:, bass.ts(0, 512)], sp[:, bass.ts(1, 512)]]
                        else:
                            sps = [
                                sps_p.tile([128, 512], F32, tag="sp", name=f"sps{i}")[:]
                                for i in range(2)
                            ]
                        for hi, off in enumerate(offs):
                            nc.tensor.matmul(
                                sps[hi],
                                kT_blk[pr][off : off + D, ksl],
                                qT_blk[pr][off : off + D, hv],
                                start=True,
                                stop=True,
                            )
                        if pack_s:
                            pt = ptp.tile([128, 1024], BF, tag="pT", name="pT")
                            pts = [pt[:, bass.ts(0, 512)], pt[:, bass.ts(1, 512)]]
                            if probe_noexp:
                                nc.vector.memset(pt[:], 0.001)
                            else:
                                nc.scalar.activation(
                                    pt[:], sp[:], mybir.ActivationFunctionType.Exp
                                )
                        else:
                            pts = []
                            for hi in range(2):
                                ptt = ptp.tile(
                                    [128, 512], BF, tag="pT", name=f"pT{hi}"
                                )
                                if probe_noexp:
                                    nc.vector.memset(ptt[:], 0.001)
                                else:
                                    nc.scalar.activation(
                                        ptt[:],
                                        sps[hi],
                                        mybir.ActivationFunctionType.Exp,
                                    )
                                pts.append(ptt[:])
                        if pend is not None:
                            emit_av(pend[0], pend[1])
                        pend = (pts, kb_)
                    emit_av(pend[0], pend[1])

                    if probe_nonorm:
                        for hi, off in enumerate(offs):
                            nc.vector.tensor_copy(
                                oT_blk[pr][off : off + D, hv], avs[hi][0:D]
                            )
                        continue
                    if pack_av:
                        ssb = rsp.tile([1, 1024], F32, tag="ssb")
                        nc.vector.tensor_copy(ssb[:], avp_t[D : D + 1, :])
                        rs = rsp.tile([1, 1024], F32, tag="rs")
                        nc.vector.reciprocal_approx_fast(rs[:], ssb[:])
                        bc = rsp.tile([D, 1024], F32, tag="bc")
                        nc.gpsimd.partition_broadcast(bc[:], rs[:])
                        for hi, off in enumerate(offs):
                            nc.vector.tensor_mul(
                                oT_blk[pr][off : off + D, hv],
                                avs[hi][0:D],
                                bc[:, bass.ts(hi, 512)],
                            )
                    else:
                        for hi, off in enumerate(offs):
                            ssb = rsp.tile([1, 512], F32, tag="ssb")
                            nc.vector.tensor_copy(ssb[:], avs[hi][D : D + 1])
                            rs = rsp.tile([1, 512], F32, tag="rs")
                            nc.vector.reciprocal_approx_fast(rs[:], ssb[:])
                            bc = rsp.tile([D, 512], F32, tag="bc")
                            nc.gpsimd.partition_broadcast(bc[:], rs[:])
                            nc.vector.tensor_mul(
                                oT_blk[pr][off : off + D, hv], avs[hi][0:D], bc[:]
                            )

            def emit_proj(b, qb):
                qsl = bass.ts(qb, 128)

                def mm_p(ps, hv, step):
                    if step < NB:
                        nc.tensor.matmul(
                            ps[:],
                            oT_blk[step][:, qsl],
                            wp_sb[:, step, hv],
                            start=(step == 0),
                            stop=bias_dve and (step == NB - 1),
                        )
                    else:
                        nc.tensor.matmul(
                            ps[:], ones_bf[:, 0:128], bo_sb[0:1, hv], start=False,
                            stop=True,
                        )

                pss = pair_chain(mm_p, NB if bias_dve else NB + 1)
                ost = outp.tile([128, N], BF, tag="ost")
                for i, hv in enumerate(HALVES):
                    if bias_dve:
                        nc.vector.tensor_add(ost[:, hv], pss[i][:], bo_sb[:, hv])
                    else:
                        nc.vector.tensor_copy(ost[:, hv], pss[i][:])
                nc.sync.dma_start(out=out_d[b, qsl, :], in_=ost[:])

            def emit_qkv_unit(b, xt_sb, j):
                emit_qk(b, xt_sb, j)
                emit_v(b, xt_sb, j)

            def body():
                nc.vector.memset(vaug0[:, :, :, D : D + 1], 1.0)
                nc.vector.memset(vaug1[:, :, :, D : D + 1], 1.0)
                xt_sb = emit_xt(0)
                for j in range(NB):
                    emit_qkv_unit(0, xt_sb, j)
                for b in range(BSH):
                    if b + 1 < BSH:
                        xt_next = emit_xt(b + 1)
                    for pr in range(H // 2):
                        attention_pair(b, pr)
                        if b + 1 < BSH and pr < NB:
                            emit_qkv_unit(b + 1, xt_next, pr)
                    for qb in range(NB):
                        emit_proj(b, qb)

            if loop_reps > 1:
                with tc.For_i(0, loop_reps, 1):
                    body()
            else:
                body()

    nc.compile()
    return nc


def _prep_shared(W_qkv, b_qkv, lora_kA, lora_kB, lora_vA, lora_vB, W_proj, b_proj):
    def bf(a):
        return np.ascontiguousarray(a).astype(BF_NP)

    W_qkv = np.asarray(W_qkv, np.float32)
    W_proj = np.asarray(W_proj, np.float32)
    lora_kA = np.asarray(lora_kA, np.float32)
    lora_kB = np.asarray(lora_kB, np.float32)
    lora_vA = np.asarray(lora_vA, np.float32)
    lora_vB = np.asarray(lora_vB, np.float32)
    b_qkv = np.asarray(b_qkv, np.float32)
    b_proj = np.asarray(b_proj, np.float32)

    # Fold LoRA into the k/v weights (fp32 on host).
    Wk_eff = W_qkv[C : 2 * C] + LSCALE * (lora_kB @ lora_kA)
    Wv_eff = W_qkv[2 * C :] + LSCALE * (lora_vB @ lora_vA)
    # Softmax rows sum to 1, so the V bias rides through attention unchanged:
    # out = attn@(xWv^T)@Wp^T + (Wp bv + bp).
    bv = b_qkv[2 * C :]
    bo = b_proj + W_proj @ bv
    return {
        "wq": bf((W_qkv[:C].T * SCALE).reshape(NB, 128, C)),
        "wk": bf(Wk_eff.T.reshape(NB, 128, C)),
        "wv": bf(Wv_eff.T.reshape(NB, 128, C)),
        "wp": bf(W_proj.T.reshape(NB, 128, C)),
        "bq": np.ascontiguousarray((b_qkv[:C] * SCALE).reshape(NB, 128).T),
        "bk": np.ascontiguousarray(b_qkv[C : 2 * C].reshape(NB, 128).T),
        "bo": bf(np.broadcast_to(bo.reshape(1, C), (128, C))),
    }


def kernel(x, W_qkv, b_qkv, lora_kA, lora_kB, lora_vA, lora_vB, W_proj, b_proj):
    nc = build_nc(loop_reps=1)
    shared = _prep_shared(
        W_qkv, b_qkv, lora_kA, lora_kB, lora_vA, lora_vB, W_proj, b_proj
    )
    x = np.asarray(x, np.float32)
    in_maps = []
    for c in range(NCORES):
        xs = x[c * BSH : (c + 1) * BSH]
        xt = (
            np.ascontiguousarray(xs.transpose(0, 2, 1))
            .astype(BF_NP)
            .reshape(BSH, NB, 128, N)
        )
        in_maps.append({"xt": xt, **shared})
    res = run_bass_kernel_spmd(nc, in_maps, list(range(NCORES)))
    return np.concatenate(
        [res.results[c]["out"].astype(np.float32) for c in range(NCORES)], axis=0
    )


# revision 6
# speedup vs baseline: 1.1916x; 1.0829x over previous
"""Fused LoRA-attention block (qkv + k/v LoRA + MHA softmax + out-proj) for
Trainium2, data-parallel over batch across 8 NeuronCores.

Per-core layout strategy (batch shard = 2 of 16):
  - Host folds the rank-64 LoRA into W_k/W_v (W + (alpha/r) B@A, fp32), folds
    the V bias and proj bias into one output bias (softmax rows sum to 1), and
    drops the K bias entirely (softmax is invariant to the per-query constant
    q.bk). Everything is pre-transposed so each matmul's contraction lands on
    SBUF partitions; matmul data bf16 (fp32 PSUM), softmax statistics fp32.
  - Q^T/K^T computed channel-major [c_out, tok]; V token-major [tok, c_out]
    with an appended ones column per head so the attention row-sum falls out
    of the P@V matmul for free (row 64 of the [65, q] PSUM tile).
  - S^T = K@Q^T per head pair with k on partitions; the two heads of a pair
    run concurrently in separate PE row groups (K=64 each) and land in one
    [128, 1024] two-bank PSUM tile, so a single ACT exp instruction covers
    both heads (softmax runs without max-subtraction, logits bounded ~|3|).
  - The attention stream is ACT-throughput-limited (exp), so the next batch's
    qkv gemm matmuls are interleaved into the attention emission at k-block
    granularity — the in-order PE queue then always has ready work while exp
    for the next AV completes. Chains for channel block j are released only
    after attention pair j has been emitted (qT/kT WAR), i.e. one slot late.
  - The per-iteration qkv prologue is software-pipelined across For_i reps:
    the loop body's last batch interleaves the *next* rep's first-batch qkv.
  - PSUM: 4 banks S (double-buffered), 2 banks AV ([65,1024], both heads),
    2 banks gemm chains. Paired N=512 chains share each stationary load by
    interleaving the two token-halves of a PSUM pair.
"""

import sys

sys.path.insert(0, "/opt/trn_rl_repo")

import ml_dtypes
import numpy as np

import concourse.bass as bass
import concourse.mybir as mybir
import concourse.tile as tile
from concourse import bacc
from concourse.bass_utils import run_bass_kernel_spmd

NCORES = 8
B, N, C = 16, 1024, 1024
H, D, R = 16, 64, 64
BSH = B // NCORES  # batches per core
NB = C // 128  # channel blocks
SCALE = D**-0.5
LSCALE = 1.0 / R
BF = mybir.dt.bfloat16
F32 = mybir.dt.float32
BF_NP = ml_dtypes.bfloat16
HALVES = (bass.ts(0, 512), bass.ts(1, 512))
UNIT_EMISSIONS = 3 * (2 * NB + 2)  # Q,K,V chains: 16 matmuls + 2 drains each


def build_nc(
    loop_reps: int = 1,
    interleave: bool = True,
    probe_noexp: bool = False,
    probe_nonorm: bool = False,
):
    nc = bacc.Bacc(None, target_bir_lowering=False, debug=False)

    xt_d = nc.dram_tensor("xt", [BSH, NB, 128, N], BF, kind="ExternalInput")
    wq_d = nc.dram_tensor("wq", [NB, 128, C], BF, kind="ExternalInput")
    wk_d = nc.dram_tensor("wk", [NB, 128, C], BF, kind="ExternalInput")
    wv_d = nc.dram_tensor("wv", [NB, 128, C], BF, kind="ExternalInput")
    wp_d = nc.dram_tensor("wp", [NB, 128, C], BF, kind="ExternalInput")
    bq_d = nc.dram_tensor("bq", [128, NB], F32, kind="ExternalInput")
    bo_d = nc.dram_tensor("bo", [128, C], BF, kind="ExternalInput")
    out_d = nc.dram_tensor("out", [BSH, N, C], BF, kind="ExternalOutput")

    with tile.TileContext(nc) as tc:
        with (
            tc.tile_pool(name="wpool", bufs=1) as wpool,
            tc.tile_pool(name="xtp", bufs=1) as xtp,
            tc.tile_pool(name="actp", bufs=1) as actp,
            tc.tile_pool(name="ptp", bufs=4) as ptp,
            tc.tile_pool(name="rsp", bufs=2) as rsp,
            tc.tile_pool(name="outp", bufs=2) as outp,
            tc.tile_pool(name="gmps", bufs=2, space="PSUM") as gmps,
            tc.tile_pool(name="sps_p", bufs=2, space="PSUM") as sps_p,
            tc.tile_pool(name="avps", bufs=1, space="PSUM") as avps,
        ):
            # ---- persistent weights ----
            wq_sb = wpool.tile([128, NB, C], BF, tag="wq")
            wk_sb = wpool.tile([128, NB, C], BF, tag="wk")
            wv_sb = wpool.tile([128, NB, C], BF, tag="wv")
            wp_sb = wpool.tile([128, NB, C], BF, tag="wp")
            for w_sb, w_d in ((wq_sb, wq_d), (wk_sb, wk_d), (wv_sb, wv_d), (wp_sb, wp_d)):
                for blk in range(NB):
                    nc.sync.dma_start(out=w_sb[:, blk, :], in_=w_d[blk])
            bq_sb = wpool.tile([128, NB], F32, tag="bq")
            nc.sync.dma_start(out=bq_sb[:], in_=bq_d[:])
            bo_sb = wpool.tile([128, C], BF, tag="bo")
            nc.sync.dma_start(out=bo_sb[:], in_=bo_d[:])

            # V with per-head ones column appended: [128, tblk, head, 65]
            vaug0 = wpool.tile([128, NB, H, D + 1], BF, tag="vaug0")
            vaug1 = wpool.tile([128, NB, H, D + 1], BF, tag="vaug1")
            vaugs = (vaug0, vaug1)
            nc.vector.memset(vaug0[:, :, :, D : D + 1], 1.0)
            nc.vector.memset(vaug1[:, :, :, D : D + 1], 1.0)
            qT_blk = [
                actp.tile([128, N], BF, tag=f"qT{cb}", name=f"qT{cb}")
                for cb in range(NB)
            ]
            kT_blk = [
                actp.tile([128, N], BF, tag=f"kT{cb}", name=f"kT{cb}")
                for cb in range(NB)
            ]
            oT_blk = [
                actp.tile([128, N], BF, tag=f"oT{cb}", name=f"oT{cb}")
                for cb in range(NB)
            ]

            def gemm_chain(emit_mm, drain, n_steps):
                """Generator: paired 512-wide PSUM chains, yield per emission."""
                pss = [
                    gmps.tile([128, 512], F32, tag="gm", name=f"gm{i}")
                    for i in range(2)
                ]
                for step in range(n_steps):
                    for i, hv in enumerate(HALVES):
                        emit_mm(pss[i], hv, step)
                        yield
                for i, hv in enumerate(HALVES):
                    drain(pss[i], hv, i)
                    yield

            def q_chain(xt_sb, cb):
                csl = bass.ts(cb, 128)

                def mm(ps, hv, ci):
                    nc.tensor.matmul(
                        ps[:],
                        wq_sb[:, ci, csl],
                        xt_sb[:, ci, hv],
                        start=(ci == 0),
                        stop=(ci == NB - 1),
                    )

                def drain(ps, hv, i):
                    nc.vector.tensor_scalar_add(
                        qT_blk[cb][:, hv], ps[:], bq_sb[:, cb : cb + 1]
                    )

                return gemm_chain(mm, drain, NB)

            def k_chain(xt_sb, cb):
                csl = bass.ts(cb, 128)

                def mm(ps, hv, ci):
                    nc.tensor.matmul(
                        ps[:],
                        wk_sb[:, ci, csl],
                        xt_sb[:, ci, hv],
                        start=(ci == 0),
                        stop=(ci == NB - 1),
                    )

                def drain(ps, hv, i):
                    nc.vector.tensor_copy(kT_blk[cb][:, hv], ps[:])

                return gemm_chain(mm, drain, NB)

            def v_chain(b, xt_sb, tb):
                vaug_sb = vaugs[b % 2]
                tsl = bass.ts(tb, 128)

                def mm(ps, hv, step):
                    nc.tensor.matmul(
                        ps[:],
                        xt_sb[:, step, tsl],
                        wv_sb[:, step, hv],
                        start=(step == 0),
                        stop=(step == NB - 1),
                    )

                def drain(ps, hv, i):
                    nc.vector.tensor_copy(
                        vaug_sb[:, tb, i * 8 : (i + 1) * 8, 0:D],
                        ps[:].rearrange("p (h d) -> p h d", d=D),
                    )

                return gemm_chain(mm, drain, NB)

            def gemm_stream(b, xt_sb):
                for j in range(NB):
                    yield from q_chain(xt_sb, j)
                    yield from k_chain(xt_sb, j)
                    yield from v_chain(b, xt_sb, j)

            class Puller:
                def __init__(self, stream):
                    self.stream = stream
                    self.pulled = 0
                    self.done = stream is None

                def pull(self, n, cap):
                    while n > 0 and not self.done and self.pulled < cap:
                        try:
                            next(self.stream)
                            self.pulled += 1
                        except StopIteration:
                            self.done = True
                        n -= 1

                def drain_all(self):
                    while not self.done:
                        try:
                            next(self.stream)
                        except StopIteration:
                            self.done = True

            def emit_xt(b):
                xt_sb = xtp.tile([128, NB, N], BF, tag="xt", name="xt")
                for blk in range(NB):
                    nc.sync.dma_start(out=xt_sb[:, blk, :], in_=xt_d[b, blk])
                return xt_sb

            def attention_pair(b, pr, puller):
                # chains for channel block j only after attention pair j:
                # during slot pr, units 0..pr-1 are eligible.
                cap = pr * UNIT_EMISSIONS
                vaug_sb = vaugs[b % 2]
                offs = (0, 64)
                for hv_i, hv in enumerate(HALVES):
                    # [65, 1024]: head a in [:, 0:512], head b in [:, 512:1024];
                    # row 64 is the softmax denominator.
                    avp_t = avps.tile([D + 1, 1024], F32, tag="avp", name="avp")
                    avs = [avp_t[:, bass.ts(0, 512)], avp_t[:, bass.ts(1, 512)]]

                    def emit_av(pts, kb_):
                        for hi in range(2):
                            h = 2 * pr + hi
                            nc.tensor.matmul(
                                avs[hi],
                                vaug_sb[:, kb_, h, :],
                                pts[hi],
                                start=(kb_ == 0),
                                stop=(kb_ == NB - 1),
                            )

                    pend = None
                    for kb_ in range(NB):
                        ksl = bass.ts(kb_, 128)
                        sp = sps_p.tile([128, 1024], F32, tag="sp", name="sp")
                        for hi, off in enumerate(offs):
                            nc.tensor.matmul(
                                sp[:, bass.ts(hi, 512)],
                                kT_blk[pr][off : off + D, ksl],
                                qT_blk[pr][off : off + D, hv],
                                start=True,
                                stop=True,
                            )
                        pt = ptp.tile([128, 1024], BF, tag="pT", name="pT")
                        pts = [pt[:, bass.ts(0, 512)], pt[:, bass.ts(1, 512)]]
                        if probe_noexp:
                            nc.vector.memset(pt[:], 0.001)
                        else:
                            nc.scalar.activation(
                                pt[:], sp[:], mybir.ActivationFunctionType.Exp
                            )
                        puller.pull(2, cap)
                        if pend is not None:
                            emit_av(pend[0], pend[1])
                        pend = (pts, kb_)
                        puller.pull(2, cap)
                    emit_av(pend[0], pend[1])

                    if probe_nonorm:
                        for hi, off in enumerate(offs):
                            nc.vector.tensor_copy(
                                oT_blk[pr][off : off + D, hv], avs[hi][0:D]
                            )
                        continue
                    ssb = rsp.tile([1, 1024], F32, tag="ssb")
                    nc.vector.tensor_copy(ssb[:], avp_t[D : D + 1, :])
                    rs = rsp.tile([1, 1024], F32, tag="rs")
                    nc.vector.reciprocal_approx_fast(rs[:], ssb[:])
                    bc = rsp.tile([D, 1024], F32, tag="bc")
                    nc.gpsimd.partition_broadcast(bc[:], rs[:])
                    for hi, off in enumerate(offs):
                        nc.vector.tensor_mul(
                            oT_blk[pr][off : off + D, hv],
                            avs[hi][0:D],
                            bc[:, bass.ts(hi, 512)],
                        )

            def emit_proj(b, qb):
                qsl = bass.ts(qb, 128)

                def mm_p(ps, hv, step):
                    nc.tensor.matmul(
                        ps[:],
                        oT_blk[step][:, qsl],
                        wp_sb[:, step, hv],
                        start=(step == 0),
                        stop=(step == NB - 1),
                    )

                pss = [
                    gmps.tile([128, 512], F32, tag="gm", name=f"gm{i}")
                    for i in range(2)
                ]
                for step in range(NB):
                    for i, hv in enumerate(HALVES):
                        mm_p(pss[i], hv, step)
                ost = outp.tile([128, N], BF, tag="ost")
                for i, hv in enumerate(HALVES):
                    nc.vector.tensor_add(ost[:, hv], pss[i][:], bo_sb[:, hv])
                nc.sync.dma_start(out=out_d[b, qsl, :], in_=ost[:])

            def emit_qkv_dense(b, xt_sb):
                p = Puller(gemm_stream(b, xt_sb))
                p.drain_all()

            def body():
                for b in range(BSH):
                    nxt = (b + 1) % BSH
                    xt_next = emit_xt(nxt)
                    puller = Puller(
                        gemm_stream(nxt, xt_next) if interleave else None
                    )
                    for pr in range(H // 2):
                        attention_pair(b, pr, puller)
                        if not interleave:
                            # v2-style: dense unit after each pair
                            if pr < NB:
                                for _ in q_chain(xt_next, pr):
                                    pass
                                for _ in k_chain(xt_next, pr):
                                    pass
                                for _ in v_chain(nxt, xt_next, pr):
                                    pass
                    puller.drain_all()
                    for qb in range(NB):
                        emit_proj(b, qb)

            # prologue: first batch's qkv, emitted once (outside the loop)
            xt0 = emit_xt(0)
            emit_qkv_dense(0, xt0)

            if loop_reps > 1:
                with tc.For_i(0, loop_reps, 1):
                    body()
            else:
                body()

    nc.compile()
    return nc


def _prep_shared(W_qkv, b_qkv, lora_kA, lora_kB, lora_vA, lora_vB, W_proj, b_proj):
    def bf(a):
        return np.ascontiguousarray(a).astype(BF_NP)

    W_qkv = np.asarray(W_qkv, np.float32)
    W_proj = np.asarray(W_proj, np.float32)
    lora_kA = np.asarray(lora_kA, np.float32)
    lora_kB = np.asarray(lora_kB, np.float32)
    lora_vA = np.asarray(lora_vA, np.float32)
    lora_vB = np.asarray(lora_vB, np.float32)
    b_qkv = np.asarray(b_qkv, np.float32)
    b_proj = np.asarray(b_proj, np.float32)

    # Fold LoRA into the k/v weights (fp32 on host).
    Wk_eff = W_qkv[C : 2 * C] + LSCALE * (lora_kB @ lora_kA)
    Wv_eff = W_qkv[2 * C :] + LSCALE * (lora_vB @ lora_vA)
    # Softmax rows sum to 1, so the V bias rides through attention unchanged:
    # out = attn@(xWv^T)@Wp^T + (Wp bv + bp). The K bias only adds a
    # per-query constant to the logits, which softmax ignores — dropped.
    bv = b_qkv[2 * C :]
    bo = b_proj + W_proj @ bv
    return {
        "wq": bf((W_qkv[:C].T * SCALE).reshape(NB, 128, C)),
        "wk": bf(Wk_eff.T.reshape(NB, 128, C)),
        "wv": bf(Wv_eff.T.reshape(NB, 128, C)),
        "wp": bf(W_proj.T.reshape(NB, 128, C)),
        "bq": np.ascontiguousarray((b_qkv[:C] * SCALE).reshape(NB, 128).T),
        "bo": bf(np.broadcast_to(bo.reshape(1, C), (128, C))),
    }


def kernel(x, W_qkv, b_qkv, lora_kA, lora_kB, lora_vA, lora_vB, W_proj, b_proj):
    nc = build_nc(loop_reps=1)
    shared = _prep_shared(
        W_qkv, b_qkv, lora_kA, lora_kB, lora_vA, lora_vB, W_proj, b_proj
    )
    x = np.asarray(x, np.float32)
    in_maps = []
    for c in range(NCORES):
        xs = x[c * BSH : (c + 1) * BSH]
        xt = (
            np.ascontiguousarray(xs.transpose(0, 2, 1))
            .astype(BF_NP)
            .reshape(BSH, NB, 128, N)
        )
        in_maps.append({"xt": xt, **shared})
    res = run_bass_kernel_spmd(nc, in_maps, list(range(NCORES)))
    return np.concatenate(
        [res.results[c]["out"].astype(np.float32) for c in range(NCORES)], axis=0
    )
